# revision 52
# baseline (speedup 1.0000x reference)
"""Bass/Trainium2 kernel for nn_KernelEdges (gnn_message_passing).

Computes A = exp((g_i + g_j - 2*Xf@Xf.T)/sigma^2) with zeroed diagonal,
broadcast to all B batch slots, where Xf = X.transpose(1,0,2).reshape(N, B*d).

Sharding: rows of the NxN pairwise matrix are split across 8 NeuronCores
(256 rows each).  Each core receives the full transposed operand
XT = Xf.T [B*d, N] in bf16, column-rotated so the core's own row-block
sits at columns 0:256 (one shared program; the stationary matmul operand
is a plain slice of the input tile).  Each core writes its [N/8, N] tile
ONCE in bf16; the host un-rotates, upcasts, applies the per-column
exp(g_j/sigma^2) factor (A factorizes as exp((g_i-2xx)/s2)*exp(g_j/s2)),
zeroes the diagonal and broadcasts to the B identical batch slots at
gather time (the batch dim of the reference output is an exact
broadcast).

Device work is a pure Gram matrix + exp:
  psum[mt, blk] = sum_q xt_q[:, mt-slice].T @ xt_q[:, blk]
  A[:, blk]     = exp(-2/sigma^2 * psum + g_i/sigma^2)    (ACT, bias/row)

Performance structure:
 - ONE packed DRAM input per core, [128, 4 + 4*N] bf16: 4 bias columns
   (the f32 g_i/sigma^2 pair, bit-viewed) then the 4 k-tiles packed
   q-major per DMA block, so each block is a single
   contiguous-per-partition DMA (dma_start instructions cost the issuing
   engine ~0.7us each, so few big DMAs beat many small ones).
 - DMA blocks stream in on the scalar ring in consumption order (the
   ring is FIFO, so arrival order is guaranteed; splitting input across
   both rings scrambles arrival order and is a net loss).
 - One PSUM bank-tile per accumulation chain: a chain stops as soon as
   its column block lands, so the exp ACTs and the output stores overlap
   the input stream.  Chains [256,512,512,512,256]; the edge chains are
   narrow so the PE starts earlier and the post-matmul tail is short.
   (A single wide PSUM tile serializes each block's matmul-start against
   the previous block's ACT read - avoid.)
 - Dummy warm-up matmuls bridge the preamble-to-first-block window so
   the HAM clock gate keeps the PE near 2.4 GHz for the real matmuls.
 - Outputs leave as 3 column-group stores; the last pair dispatches from
   the scalar engine right after the final ACT.
"""

import numpy as np

B, N, D = 8, 2048, 64
NCORES = 8
R = N // NCORES          # 256 rows per core
KD = B * D               # 512 contraction dim
NMT = R // 128           # 2 m-tiles per core
NQ = KD // 128           # 4 k-tiles
BW = [256, 512, 512, 256, 256, 256]     # chain (PSUM-group) widths
BC = [0, 256, 768, 1280, 1536, 1792]    # chain column starts
NBLK = len(BW)
# One DMA block per chain: each chain's matmuls unblock as soon as its
# own transfer lands (a merged 1MB middle block left the PE idle ~1.8us
# waiting on the combined completion semaphore).  The trailing blocks are
# 256 wide: each DMA's ~1.5-2us completion receipt pipelines at ~0.5us
# spacing, compressing the post-input matmul tail.
DBW = [256, 512, 512, 256, 256, 256]    # DMA-block widths
DBC = [0, 256, 768, 1280, 1536, 1792]   # DMA-block column starts
DBI = [0, 1, 2, 3, 4, 5]                # chain -> DMA block
DOF = [0, 0, 0, 0, 0, 0]                # chain offset inside its DMA block
MM_DT = "bf16"           # matmul operand dtype: "fp8" (e4m3) or "bf16"
ELEM = 1 if MM_DT == "fp8" else 2       # bytes per packed input element
OFF = 8 // ELEM          # bias columns (8 bytes of f32 pair) at the head
NWARM = 10               # PE warm-up matmuls (~3.4us at cold clock)


def _build_program(inv_s2):
    import concourse.tile as tile
    from concourse import bacc, mybir

    f32 = mybir.dt.float32
    bf16 = mybir.dt.bfloat16
    mdt = mybir.dt.float8e4 if MM_DT == "fp8" else bf16
    mdt_u = mybir.dt.uint8 if MM_DT == "fp8" else mybir.dt.uint16

    nc = bacc.Bacc(
        "TRN2", target_bir_lowering=False, debug=False, num_devices=NCORES
    )

    xt_d = nc.dram_tensor(
        "xt2", [128, OFF + NQ * N], mdt, kind="ExternalInput"
    ).ap()
    out_d = nc.dram_tensor("out", [R, N], bf16, kind="ExternalOutput").ap()

    # packed column start of each DMA block
    DS = [OFF + NQ * c for c in DBC]

    with tile.TileContext(nc) as tc:
        with (
            tc.tile_pool(name="persist", bufs=1) as persist,
            tc.tile_pool(name="apool", bufs=1) as apool,
            tc.tile_pool(name="psum", bufs=1, space="PSUM") as pspool,
        ):
            neg_half = persist.tile([2, 256], mdt, name="warmops")
            nc.vector.memset(
                neg_half[:].bitcast(mdt_u),
                0xB0 if MM_DT == "fp8" else 0xBF00,
            )

            xt_all = persist.tile([128, OFF + NQ * N], mdt, name="xt")
            # all input blocks on the sync ring IN CONSUMPTION ORDER (the
            # ring is FIFO, so arrival order is guaranteed); block 0
            # carries the bias columns and the stationary slices.  Outputs
            # dispatch from the scalar engine (idle slack between ACTs),
            # so they never queue behind input blocks.
            for i in range(len(DBW)):
                lo = DS[i] - (OFF if i == 0 else 0)
                hi = DS[i] + NQ * DBW[i]
                nc.sync.dma_start(xt_all[:, lo:hi], xt_d[:, lo:hi])

            bias_ap = xt_all[:, 0:OFF].bitcast(f32)

            # one PSUM bank-tile per accumulation chain so the PE never
            # waits for an ACT to drain a bank (a shared big tile serializes
            # matmul-start against the previous block's ACT read).  The two
            # 256-wide edge blocks (0 and 4) share one bank per mt; their
            # lifetimes don't overlap.
            edge = {
                mt: pspool.tile([128, 512], f32, name=f"pse{mt}")
                for mt in range(NMT)
            }
            mid = {
                (mt, b): pspool.tile([128, 512], f32, name=f"ps{mt}{b}")
                for mt in range(NMT)
                for b in (1, 2)
            }
            # chains 3 and 4 (256 wide each) share one bank per mt, like
            # the edge chains; their slices are disjoint
            m34 = {
                mt: pspool.tile([128, 512], f32, name=f"ps34{mt}")
                for mt in range(NMT)
            }

            def ps_ap(mt, b):
                if b == 0:
                    return edge[mt][:, 0:256]
                if b == NBLK - 1:
                    return edge[mt][:, 256:512]
                if b == 3:
                    return m34[mt][:, 0:256]
                if b == 4:
                    return m34[mt][:, 256:512]
                return mid[mt, b][:]

            # PE warm-up: dummy matmuls (results discarded) so the HAM
            # clock gate lifts the PE to 2.4 GHz before the real work.
            for w in range(NWARM):
                nc.tensor.matmul(
                    mid[w % NMT, 1 + w % 2][:, 0:256],
                    neg_half[:, 0:128],
                    neg_half[:],
                    start=True,
                    stop=True,
                )

            # one [128, NMT*N] tile (mt-major) so each output column group
            # leaves as a single two-run DMA covering both m-tiles
            a_all = apool.tile([128, NMT * N], bf16, name="a")
            a_src = a_all[:].rearrange("p (m n) -> p m n", m=NMT)
            o_dst = out_d.rearrange("(m p) n -> p m n", p=128)
            for b in range(NBLK):
                c, w = BC[b], BW[b]
                s = DS[DBI[b]]
                rw = DBW[DBI[b]]
                for q in range(NQ):
                    for mt in range(NMT):
                        # rotated layout: this core's own rows are the
                        # 256 data columns of block 0's q-runs
                        nc.tensor.matmul(
                            ps_ap(mt, b),
                            xt_all[:, OFF + q * DBW[0] + mt * 128:
                                   OFF + q * DBW[0] + (mt + 1) * 128],
                            xt_all[:, s + q * rw + DOF[b]:
                                   s + q * rw + DOF[b] + w],
                            start=q == 0,
                            stop=q == NQ - 1,
                        )
                for mt in range(NMT):
                    nc.scalar.activation(
                        a_all[:, mt * N + c:mt * N + c + w],
                        ps_ap(mt, b),
                        mybir.ActivationFunctionType.Exp,
                        bias=bias_ap[:, mt:mt + 1],
                        scale=-2.0 * inv_s2,
                    )
                # store groups, in columns [0:1024) / [1024:1792) /
                # [1792:2048).  The last group dispatches from the scalar
                # engine itself: it directly follows the final ACT there,
                # skipping a cross-engine hop.
                # store groups aligned to chain completion so the output
                # stream overlaps the input stream instead of trailing it:
                # cols [0:768) are done right after block 1's ACTs.
                og = {
                    1: (0, 768),
                    2: (768, 1280),
                    4: (1280, 1792),
                    5: (1792, 2048),
                }
                if b in og:
                    o0, o1 = og[b]
                    nc.scalar.dma_start(o_dst[:, :, o0:o1], a_src[:, :, o0:o1])

    nc.compile()
    return nc


def _prepare(X, log_sigma):
    """Host prep: returns (inv_s2, g, in_maps) for run_bass_kernel_spmd."""
    import ml_dtypes

    X = np.ascontiguousarray(X, dtype=np.float32)
    assert X.shape == (B, N, D), X.shape

    sigma = float(np.exp(np.float32(log_sigma)))
    inv_s2 = 1.0 / (sigma * sigma)

    # XT[b*D+f, n] = X[b, n, f]
    XT = np.ascontiguousarray(X.transpose(0, 2, 1).reshape(KD, N))
    g = np.einsum("kn,kn->n", XT, XT).astype(np.float32)  # [N]
    pdt = ml_dtypes.float8_e4m3fn if MM_DT == "fp8" else ml_dtypes.bfloat16
    XTb = XT.astype(pdt)

    in_maps = []
    for cix in range(NCORES):
        r0 = cix * R
        # rotate columns so this core's own rows land at cols 0:R
        XTr = np.roll(XTb, -r0, axis=1)
        packed = np.empty((128, OFF + NQ * N), dtype=pdt)
        bias_np = np.empty((128, NMT), dtype=np.float32)
        for mt in range(NMT):
            bias_np[:, mt] = g[r0 + mt * 128: r0 + (mt + 1) * 128] * inv_s2
        packed[:, 0:OFF] = bias_np.view(pdt)
        for i in range(len(DBW)):
            c, w = DBC[i], DBW[i]
            s = OFF + NQ * c
            sub = XTr[:, c:c + w].reshape(NQ, 128, w).transpose(1, 0, 2)
            packed[:, s:s + NQ * w] = sub.reshape(128, NQ * w)
        in_maps.append({"xt2": packed})
    return inv_s2, g, in_maps


def kernel(X, log_sigma):
    from concourse.bass_utils import run_bass_kernel_spmd

    inv_s2, g, in_maps = _prepare(X, log_sigma)
    nc = _build_program(inv_s2)
    res = run_bass_kernel_spmd(nc, in_maps, list(range(NCORES)))
    rows = []
    for c in range(NCORES):
        t = res.results[c]["out"].astype(np.float32)  # [R, N], rotated cols
        rows.append(np.roll(t, c * R, axis=1))
    A = np.concatenate(rows, axis=0)  # [N, N] = exp((g_i - 2*x_i.x_j)/s2)
    A *= np.exp(g * inv_s2)[None, :]  # per-column exp(g_j/s2) factor
    idx = np.arange(N)
    A[idx, idx] = 0.0
    return np.ascontiguousarray(np.broadcast_to(A[None, :, :], (B, N, N)))
